# revision 1
# baseline (speedup 1.0000x reference)
"""AdaptiveGeometryAttention Trainium2 kernel (8 NeuronCores).

Sharding: core c handles batch b = c//4 and head group hg = c%4 (4 of 16 heads).
Each core computes its heads' attention and a partial out-projection (T, E);
the host sums the 4 partials per batch.

Key algebraic restructurings vs the reference:
  - The Lorentz inner product -<q_hyp, k_hyp>_L is a single K=65 matmul over
    augmented vectors [-gf*q_d | tim] x [gf*k_d | tim] with the k-side row 0
    zeroed in the broadcast selector (cancels the discarded r0 tangent row).
  - arccosh(m)^2 ~= A*tanh(sf*(m-1)+bf) + a*(m-1) + const. Constants and
    per-query additive terms cancel in softmax and are dropped; the blend is
    computed re-centered as G = psU - gamma*(F-1) so exp(G) stays in fp16
    range. The a*mdot linear term and the (1-alpha)/8 euclid scale fold into
    a second accumulated matmul (per-query column scaling of q-side operands).
  - Softmax denominators come from a ones-column appended to V; spike masking
    and 1/Z fold into one per-query scale applied to y^T before out-proj.
  - Emission is software-pipelined: psY(group g) is emitted after the score
    matmuls of group g+1 so the PE never waits on the elementwise chain.
All big matmuls run fp32r (full-rate) except attn@V / out-proj (fp16).
"""

import sys
import contextlib

sys.path.insert(0, "/opt/trn_rl_repo")

import numpy as np

B, T, E, H = 2, 1024, 1024, 16
D = 64
NCORES = 8
HPC = 4  # heads per core

# arccosh(1+x)^2 fit on x in [0, 2.2]
A_FIT = 54.32641203
S_FIT = 0.28607594936708863
B_FIT = 2.0
A_LIN = 0.8910533
# sqrt(1+w) deg-3 fit on w in [0, 0.95]
SQ3, SQ2, SQ1, SQ0 = 0.02492195, -0.10732602, 0.49672154, 1.00014421

_DEBUG = False

# host-derived scalars, set by kernel() before _build()
_S2 = _S2M2 = _SS = _THR = _BADJ = 0.0


def _build():
    from concourse import bass, mybir, tile, bacc

    F32 = mybir.dt.float32
    F32R = mybir.dt.float32r
    F16 = mybir.dt.float16
    BF16 = mybir.dt.bfloat16
    AF = mybir.ActivationFunctionType
    OP = mybir.AluOpType

    def r(ap):
        return ap.bitcast(F32R)

    nc = bacc.Bacc()

    xT = nc.declare_dram_parameter("xT", [E, T], F32R, isOutput=False)
    wqk = nc.declare_dram_parameter("wqk", [E, 512], F32R, isOutput=False)
    wv = nc.declare_dram_parameter("wv", [E, 256], F32R, isOutput=False)
    wai = nc.declare_dram_parameter("wai", [E, 5], F32R, isOutput=False)
    bqk = nc.declare_dram_parameter("bqk", [128, 4], F32, isOutput=False)
    bqk2 = nc.declare_dram_parameter("bqk2", [64, 8], F32, isOutput=False)
    bvT = nc.declare_dram_parameter("bvT", [1, 256], F32R, isOutput=False)
    wo = nc.declare_dram_parameter("wo", [256, E], F16, isOutput=False)
    onesel = nc.declare_dram_parameter("onesel", [128, 4, 40], F32R, isOutput=False)
    cst = nc.declare_dram_parameter("cst", [5, 1], F32, isOutput=False)
    csta = nc.declare_dram_parameter("csta", [4, 1], F32, isOutput=False)
    cstl = nc.declare_dram_parameter("cstl", [4, 1], F32, isOutput=False)
    tri = nc.declare_dram_parameter("tri", [128, 128], BF16, isOutput=False)
    sel8 = nc.declare_dram_parameter("sel8", [8, 512], F32R, isOutput=False)
    selb4 = nc.declare_dram_parameter("selb4", [4, 12 * 128], F32R, isOutput=False)
    out = nc.declare_dram_parameter("out", [T, E], F32, isOutput=True)
    if _DEBUG:
        d_gf = nc.declare_dram_parameter("d_gf", [8, T], F32R, isOutput=True)
        d_tim = nc.declare_dram_parameter("d_tim", [8, T], F32R, isOutput=True)
        d_qhkh = nc.declare_dram_parameter("d_qhkh", [65, 8, T], F32R, isOutput=True)
        d_y0 = nc.declare_dram_parameter("d_y0", [128, T], F16, isOutput=True)
        d_spk = nc.declare_dram_parameter("d_spk", [1, T], F32, isOutput=True)
        d_ba = nc.declare_dram_parameter("d_ba", [4, 3, T], F32R, isOutput=True)
        d_q0 = nc.declare_dram_parameter("d_q0", [64, T], F32R, isOutput=True)
        d_k0 = nc.declare_dram_parameter("d_k0", [64, T], F32R, isOutput=True)
        d_v = nc.declare_dram_parameter("d_v", [128, 8, HPC, 65], BF16, isOutput=True)

    with tile.TileContext(nc) as tc:
        ctx = contextlib.ExitStack()
        with ctx:
            main = ctx.enter_context(tc.tile_pool(name="main", bufs=1))

            # ---- persistent small inputs (sync queue, before big weights) ----
            tSEL = main.tile([128, 4, 40], F32R)
            nc.sync.dma_start(out=tSEL[:], in_=onesel[:])
            tBQK = main.tile([128, 4], F32)
            nc.sync.dma_start(out=tBQK[:], in_=bqk[:])
            tBQK2 = main.tile([64, 8], F32)
            nc.sync.dma_start(out=tBQK2[:], in_=bqk2[:])
            tCST = main.tile([5, 1], F32)
            nc.sync.dma_start(out=tCST[:], in_=cst[:])
            tCA = main.tile([4, 1], F32)
            nc.sync.dma_start(out=tCA[:], in_=csta[:])
            tCL = main.tile([4, 1], F32)
            nc.sync.dma_start(out=tCL[:], in_=cstl[:])
            tBVT = main.tile([1, 256], F32R)
            nc.sync.dma_start(out=tBVT[:], in_=bvT[:])
            tS8 = main.tile([8, 512], F32R)
            nc.sync.dma_start(out=tS8[:], in_=sel8[:])
            tS4 = main.tile([4, 12 * 128], F32R)
            nc.sync.dma_start(out=tS4[:], in_=selb4[:])
            tTRI = main.tile([128, 128], BF16)
            nc.sync.dma_start(out=tTRI[:], in_=tri[:])

            tONEf = main.tile([1, 128], F32)
            nc.vector.memset(tONEf[:], 1.0)
            tONE = main.tile([1, 128], F32R)
            nc.vector.tensor_copy(out=tONE[:], in_=tONEf[:])
            tBADJ = main.tile([128, 1], F32)
            nc.vector.memset(tBADJ[:], _BADJ)
            tC625 = main.tile([4, 1], F32)
            nc.vector.memset(tC625[:], 0.0625)

            # ---- persistent state ----
            tQ = [main.tile([64, T], F32R, name=f"tQ{h}") for h in range(HPC)]
            tK = [main.tile([64, T], F32R, name=f"tK{h}") for h in range(HPC)]
            tV = main.tile([128, 8, HPC, 65], BF16)
            QHKH = main.tile([65, 8, T], F32R)  # slot 2h = QH_h, 2h+1 = KH_h
            Pgf = main.tile([8, T], F32R)
            Ptim = main.tile([8, T], F32R)
            RW = main.tile([8, 4, T], F32)
            tA = main.tile([5, T], F32)
            beta = main.tile([4, T], F32R)
            gA = main.tile([4, T], F32R)
            ag = main.tile([4, T], F32R)
            SPK5 = main.tile([5, T], F32)
            tY0 = main.tile([128, T], F16)
            tY1 = main.tile([128, T], F16)
            tYL = [tY0, tY1]
            tWO = main.tile([128, 2, E], F16)
            for g in range(2):
                nc.sync.dma_start(out=tWO[:, g, :], in_=wo[g * 128:(g + 1) * 128, :])

            # ================= projection phase =================
            with tc.tile_pool(name="pin", bufs=1) as pin, \
                 tc.tile_pool(name="ppj2", bufs=1, space="PSUM") as ppj2:
                tXT = pin.tile([128, 8, T], F32R)
                tWQK = pin.tile([128, 8, 512], F32R)
                tWV = pin.tile([128, 8, 256], F32R)
                tWAI = pin.tile([128, 8, 5], F32R)
                for k in range(8):
                    nc.gpsimd.dma_start(out=tXT[:, k, :],
                                        in_=xT[k * 128:(k + 1) * 128, :])
                    nc.sync.dma_start(out=tWQK[:, k, :],
                                      in_=wqk[k * 128:(k + 1) * 128, :])
                for k in range(8):
                    nc.sync.dma_start(out=tWV[:, k, :],
                                      in_=wv[k * 128:(k + 1) * 128, :])
                    nc.scalar.dma_start(out=tWAI[:, k, :],
                                        in_=wai[k * 128:(k + 1) * 128, :])

                psNZ = ppj2.tile([40, T], F32, tag="nz")
                with tc.tile_pool(name="ppj", bufs=2, space="PSUM") as ppj:
                    for h in range(HPC):
                        ps = ppj.tile([128, T], F32, tag="psqk")
                        for k in range(8):
                            for n in range(2):
                                nc.tensor.matmul(
                                    ps[:, n * 512:(n + 1) * 512],
                                    r(tWQK[:, k, h * 128:(h + 1) * 128]),
                                    r(tXT[:, k, n * 512:(n + 1) * 512]),
                                    start=(k == 0), stop=(k == 7),
                                )
                        sq = pin.tile([128, T], F32R, tag="sq", bufs=2)
                        nc.scalar.activation(out=sq[:], in_=ps[:], func=AF.Square,
                                             bias=tBQK[:, h:h + 1])
                        nc.scalar.activation(out=tQ[h][:], in_=ps[0:64, :],
                                             func=AF.Identity,
                                             bias=tBQK2[:, h:h + 1])
                        nc.vector.tensor_scalar(tK[h][:], ps[64:128, :],
                                                tBQK2[:, 4 + h:5 + h], None,
                                                op0=OP.add)
                        for n in range(2):
                            nc.tensor.matmul(
                                psNZ[:, n * 512:(n + 1) * 512],
                                r(tSEL[:, h, 0:40]),
                                r(sq[:, n * 512:(n + 1) * 512]),
                                start=(h == 0), stop=(h == HPC - 1),
                            )

                psA = ppj2.tile([5, T], F32, tag="alpha")
                for k in range(8):
                    for n in range(2):
                        nc.tensor.matmul(
                            psA[:, n * 512:(n + 1) * 512],
                            r(tWAI[:, k, :]),
                            r(tXT[:, k, n * 512:(n + 1) * 512]),
                            start=(k == 0), stop=(k == 7),
                        )

                nc.scalar.activation(out=tA[:], in_=psA[:], func=AF.Tanh,
                                     scale=0.5, bias=tCST[0:5, :])
                nc.vector.tensor_scalar(SPK5[:], psA[0:5, :], _THR, None,
                                        op0=OP.is_gt)
                nc.scalar.activation(out=beta[:], in_=tA[0:4, :], func=AF.Identity,
                                     scale=-0.0625, bias=tC625[:])
                nc.scalar.activation(out=gA[:], in_=tA[0:4, :], func=AF.Identity,
                                     scale=tCA[:], bias=tCA[:])
                nc.scalar.activation(out=ag[:], in_=tA[0:4, :], func=AF.Identity,
                                     scale=tCL[:], bias=tCL[:])
                spk = main.tile([1, T], F32)
                nc.scalar.dma_start(out=spk[:], in_=SPK5[4:5, :])

                # ---- row quantities, chunk cc (serial chain on vector) ----
                def row_chunk(cc):
                    cl = slice(cc * 512, (cc + 1) * 512)
                    sA = RW[0:8, 0, cl]
                    sB = RW[0:8, 1, cl]
                    sC = RW[0:8, 2, cl]
                    sD = RW[0:8, 3, cl]
                    gfc = Pgf[0:8, cl]
                    timc = Ptim[0:8, cl]
                    n2 = psNZ[0:8, cl]
                    z2 = psNZ[32:40, cl]
                    nc.vector.tensor_scalar_max(sA, n2, 1e-24)
                    nc.vector.reciprocal_approx_fast(out=sB, in_=sA)      # 1/n2
                    nc.vector.tensor_mul(sC, z2, sB)                      # q2n
                    nc.vector.tensor_scalar(sA, sC, _S2M2, _S2, op0=OP.mult,
                                            op1=OP.add)
                    nc.vector.tensor_scalar_max(sA, sA, 1e-8)             # y = nu^2
                    nc.scalar.activation(out=gfc, in_=sB, func=AF.Sqrt)   # invn
                    # f = sinh(nu)/nu = 1 + y/6 + y^2/120 + y^3/5040
                    nc.vector.tensor_scalar(sD, sA, 1.0 / 5040.0, 1.0 / 120.0,
                                            op0=OP.mult, op1=OP.add)
                    nc.vector.tensor_mul(sD, sD, sA)
                    nc.vector.scalar_tensor_tensor(out=sD, in0=sD, scalar=1.0 / 6.0,
                                                   in1=sA, op0=OP.add, op1=OP.mult)
                    nc.vector.tensor_scalar_add(sA, sD, 1.0)              # f
                    nc.vector.scalar_tensor_tensor(out=gfc, in0=gfc, scalar=_SS,
                                                   in1=sA, op0=OP.mult, op1=OP.mult)
                    nc.vector.tensor_scalar(sB, sC, -_S2, _S2, op0=OP.mult,
                                            op1=OP.add)
                    nc.vector.tensor_mul(sC, sA, sA)                      # f^2
                    nc.vector.tensor_mul(sB, sC, sB)                      # w
                    nc.vector.tensor_scalar(sC, sB, SQ3, SQ2, op0=OP.mult,
                                            op1=OP.add)
                    nc.vector.tensor_mul(sC, sC, sB)
                    nc.vector.scalar_tensor_tensor(out=sC, in0=sC, scalar=SQ1,
                                                   in1=sB, op0=OP.add, op1=OP.mult)
                    nc.vector.tensor_scalar_add(timc, sC, SQ0)            # time
                    # time rows into QHKH row 64 (one DMA off the engines)
                    nc.scalar.dma_start(out=QHKH[64:65, :, cl], in_=Ptim[0:8, cl])

                row_chunk(0)

                # V projection (k-outer, PSUM-resident accumulators)
                tVonef = pin.tile([128, 32], F32)
                nc.vector.memset(tVonef[:], 1.0)
                nc.vector.tensor_copy(out=tV[:, :, :, 64:65], in_=tVonef[:])
                with tc.tile_pool(name="ppv", bufs=1, space="PSUM") as ppv:
                    psvAll = ppv.tile([128, 8, 256], F32, tag="psv")
                    for m in range(8):
                        nc.tensor.matmul(psvAll[:, m, :], r(tONE[:]), r(tBVT[:]),
                                         start=True, stop=False)
                    for k in range(8):
                        for m in range(8):
                            nc.tensor.matmul(
                                psvAll[:, m, :],
                                r(tXT[:, k, m * 128:(m + 1) * 128]),
                                r(tWV[:, k, :]),
                                start=False, stop=(k == 7),
                            )
                    for m in range(8):
                        src = psvAll[:, m, :].rearrange("p (h d) -> p h d", h=HPC)
                        nc.scalar.copy(out=tV[:, m, :, 0:64], in_=src)

                    row_chunk(1)

            # ================= attention =================
            jsl = [slice(0, 512), slice(512, 1024)]
            with tc.tile_pool(name="hp", bufs=2) as hp, \
                 tc.tile_pool(name="pps", bufs=2, space="PSUM") as pps, \
                 tc.tile_pool(name="ppy", bufs=2, space="PSUM") as ppy, \
                 tc.tile_pool(name="ppb", bufs=2, space="PSUM") as ppb:

                BAG = {}

                def prep(h, cc):
                    csl = slice(cc * 512, (cc + 1) * 512)
                    if cc == 0:
                        BAG[h] = (
                            hp.tile([64, T], F32R, name=f"BQ{h}", tag="BQ"),
                            hp.tile([65, T], F32R, name=f"AGQ{h}", tag="AGQ"),
                            hp.tile([128, T], F16, name=f"GAB{h}", tag="GAB"),
                        )
                    tbq, tagq, tgab = BAG[h]
                    pb = ppb.tile([128, 512], F32, tag="psb")
                    nc.tensor.matmul(pb[:], r(tS8[:, h * 128:(h + 1) * 128]),
                                     r(Pgf[:, csl]), start=True, stop=True)
                    nc.vector.tensor_mul(QHKH[0:64, 2 * h, csl], pb[0:64, :],
                                         tQ[h][:, csl])
                    nc.vector.tensor_mul(QHKH[0:64, 2 * h + 1, csl], pb[64:128, :],
                                         tK[h][:, csl])
                    pb2 = ppb.tile([128, 512], F32, tag="psb")
                    nc.tensor.matmul(pb2[0:64, :],
                                     r(tS4[:, h * 128:h * 128 + 64]),
                                     r(beta[:, csl]), start=True, stop=True)
                    nc.vector.tensor_mul(tbq[:, csl], pb2[0:64, :], tQ[h][:, csl])
                    pb3 = ppb.tile([128, 512], F32, tag="psb")
                    nc.tensor.matmul(pb3[0:65, :],
                                     r(tS4[:, (8 + h) * 128:(8 + h) * 128 + 65]),
                                     r(ag[:, csl]), start=True, stop=True)
                    nc.vector.tensor_mul(tagq[:, csl], pb3[0:65, :],
                                         QHKH[0:65, 2 * h, csl])
                    pb4 = ppb.tile([128, 512], F32, tag="psb")
                    nc.tensor.matmul(pb4[:], r(tS4[:, (4 + h) * 128:(5 + h) * 128]),
                                     r(gA[:, csl]), start=True, stop=True)
                    nc.scalar.copy(out=tgab[:, csl], in_=pb4[:])

                def scores(h, j):
                    nsb = 4 * (j + 1)
                    tbq, tagq, tgab = BAG[h]
                    PTJ = hp.tile([128, 8, 512], BF16, tag="PTJ")
                    for sb in range(nsb):
                        o = max(0, 128 * sb - 512 * j)
                        W = 512 - o
                        c0 = 512 * j + o
                        psM = pps.tile([128, 512], F32, tag="psM")
                        psU = pps.tile([128, 512], F32, tag="psU")
                        nc.tensor.matmul(
                            psM[:, o:512],
                            r(QHKH[0:65, 2 * h + 1, sb * 128:(sb + 1) * 128]),
                            r(QHKH[0:65, 2 * h, c0:c0 + W]),
                            start=True, stop=True)
                        nc.tensor.matmul(
                            psU[:, o:512],
                            r(tK[h][:, sb * 128:(sb + 1) * 128]),
                            r(tbq[:, c0:c0 + W]), start=True, stop=False)
                        nc.tensor.matmul(
                            psU[:, o:512],
                            r(QHKH[0:65, 2 * h + 1, sb * 128:(sb + 1) * 128]),
                            r(tagq[:, c0:c0 + W]), start=False, stop=True)
                        F = hp.tile([128, 512], F16, tag="F", bufs=3)
                        nc.scalar.activation(out=F[:, o:512], in_=psM[:, o:512],
                                             func=AF.Tanh, scale=S_FIT,
                                             bias=tBADJ[:])
                        G = hp.tile([128, 512], F16, tag="G", bufs=3)
                        # G = (F - 1) * gab  (re-centered: shifts scores by
                        # +gamma(q), a per-query constant that softmax cancels)
                        nc.gpsimd.tensor_mul(G[:, o:512], F[:, o:512],
                                             tgab[:, c0:c0 + W])
                        nc.vector.scalar_tensor_tensor(
                            out=G[:, o:512], in0=G[:, o:512], scalar=-1.0,
                            in1=psU[:, o:512], op0=OP.mult, op1=OP.add)
                        nc.scalar.activation(out=PTJ[:, sb, o:512], in_=G[:, o:512],
                                             func=AF.Exp)
                        if sb >= 4 * j:
                            nc.gpsimd.tensor_mul(PTJ[:, sb, o:o + 128],
                                                 PTJ[:, sb, o:o + 128], tTRI[:, :])
                    return PTJ

                def finishA(h, j, PTJ):
                    nsb = 4 * (j + 1)
                    psY = ppy.tile([65, 512], F32, tag="psY")
                    for sb in range(nsb):
                        o = max(0, 128 * sb - 512 * j)
                        nc.tensor.matmul(
                            psY[:, o:512],
                            tV[:, sb, h, :],
                            PTJ[:, sb, o:512],
                            start=(sb == 0), stop=(sb == nsb - 1))
                    zrow = hp.tile([1, 512], F32, tag="zrow", bufs=1)
                    nc.vector.tensor_copy(out=zrow[:], in_=psY[64:65, :])
                    rz = hp.tile([1, 512], F32, tag="rz")
                    nc.vector.reciprocal_approx_fast(out=rz[:], in_=zrow[:])
                    cs = hp.tile([1, 512], F32R, tag="cs")
                    nc.vector.tensor_mul(cs[:], rz[:], spk[0:1, jsl[j]])
                    return psY, cs

                def finishB(h, j, psY, cs):
                    psc = pps.tile([128, 512], F32, tag="psM")
                    nc.tensor.matmul(psc[0:64, :], r(tONE[:, 0:64]), r(cs[:]),
                                     start=True, stop=True)
                    cbs = hp.tile([64, 512], F32, tag="cbs", bufs=1)
                    nc.scalar.copy(out=cbs[:], in_=psc[0:64, :])
                    g = h // 2
                    rows = slice((h % 2) * 64, (h % 2) * 64 + 64)
                    nc.vector.tensor_mul(tYL[g][rows, jsl[j]], psY[0:64, :],
                                         cbs[:])

                order = [(0, 0), (1, 0), (0, 1), (1, 1),
                         (2, 0), (3, 0), (2, 1), (3, 1)]
                preps = {0: [(0, 1)], 1: [(1, 1)], 2: [(2, 0)], 3: [(3, 0)],
                         4: [(2, 1)], 5: [(3, 1)]}
                prep(0, 0)
                prep(1, 0)
                PTs = {}
                Ys = {}
                for gi, (h, j) in enumerate(order):
                    PTs[gi] = scores(h, j)
                    for (ph, pc) in preps.get(gi, []):
                        prep(ph, pc)
                    if gi >= 1:
                        h1, j1 = order[gi - 1]
                        Ys[gi - 1] = finishA(h1, j1, PTs.pop(gi - 1))
                    if gi >= 2:
                        h2, j2 = order[gi - 2]
                        finishB(h2, j2, *Ys.pop(gi - 2))
                h1, j1 = order[7]
                Ys[7] = finishA(h1, j1, PTs.pop(7))
                h2, j2 = order[6]
                finishB(h2, j2, *Ys.pop(6))
                finishB(h1, j1, *Ys.pop(7))

                if _DEBUG:
                    nc.sync.dma_start(out=d_y0[:], in_=tY0[:])
                    nc.sync.dma_start(out=d_gf[:], in_=Pgf[:])
                    nc.sync.dma_start(out=d_tim[:], in_=Ptim[:])
                    nc.sync.dma_start(out=d_qhkh[:], in_=QHKH[:])
                    nc.sync.dma_start(out=d_spk[:], in_=spk[:])
                    nc.sync.dma_start(out=d_ba[:, 0, :], in_=beta[:])
                    nc.sync.dma_start(out=d_ba[:, 1, :], in_=gA[:])
                    nc.sync.dma_start(out=d_ba[:, 2, :], in_=ag[:])
                    nc.sync.dma_start(out=d_q0[:], in_=tQ[0][:])
                    nc.sync.dma_start(out=d_k0[:], in_=tK[0][:])
                    nc.sync.dma_start(out=d_v[:], in_=tV[:])

                # ---- out projection: partial (T, E) ----
                for m in range(8):
                    po = pps.tile([128, 512], F32, tag="psM")
                    po2 = pps.tile([128, 512], F32, tag="psU")
                    for ne, pot in ((0, po), (1, po2)):
                        for g in range(2):
                            nc.tensor.matmul(
                                pot[:],
                                tYL[g][:, m * 128:(m + 1) * 128],
                                tWO[:, g, ne * 512:(ne + 1) * 512],
                                start=(g == 0), stop=(g == 1))
                    oo = hp.tile([128, 512], F32, tag="oo")
                    oo2 = hp.tile([128, 512], F32, tag="oo2")
                    nc.scalar.copy(out=oo[:], in_=po[:])
                    nc.scalar.copy(out=oo2[:], in_=po2[:])
                    eng = nc.sync if m % 2 == 0 else nc.gpsimd
                    eng.dma_start(out=out[m * 128:(m + 1) * 128, 0:512], in_=oo[:])
                    eng.dma_start(out=out[m * 128:(m + 1) * 128, 512:1024],
                                  in_=oo2[:])

    nc.finalize()
    return nc


_NC_CACHE = None


def _np_sigmoid(x):
    return 1.0 / (1.0 + np.exp(-x))


def kernel(**inputs):
    global _NC_CACHE, _S2, _S2M2, _SS, _THR, _BADJ
    x = np.asarray(inputs["x"], np.float32)
    Wqkv = np.asarray(inputs["Wqkv"], np.float32)
    bqkv = np.asarray(inputs["bqkv"], np.float32)
    Wout = np.asarray(inputs["Wout"], np.float32)
    bout = np.asarray(inputs["bout"], np.float32)
    Wimp = np.asarray(inputs["Wimp"], np.float32)
    bimp = np.asarray(inputs["bimp"], np.float32)
    Walpha = np.asarray(inputs["Walpha"], np.float32)
    balpha = np.asarray(inputs["balpha"], np.float32)
    spike_threshold = float(np.asarray(inputs["spike_threshold"]))
    log_k = np.asarray(inputs["log_k"], np.float32)
    qk_scale = float(np.asarray(inputs["qk_scale"]))

    s = _np_sigmoid(qk_scale) * 1.5
    kh = np.log1p(np.exp(log_k.astype(np.float64))) + 1e-6
    _S2 = float(s * s)
    _S2M2 = float(-2.0 * s * s)
    _SS = float(s)
    _THR = float(np.log(spike_threshold / (1.0 - spike_threshold)) - bimp[0])
    _BADJ = float(B_FIT - S_FIT)  # tanh(sf*M + (bf - sf)) = tanh(sf*(M-1)+bf)

    if _NC_CACHE is None:
        _NC_CACHE = _build()
    nc = _NC_CACHE

    onesel = np.zeros((128, 4, 40), np.float32)
    for h in range(HPC):
        onesel[0:64, h, 2 * h] = 1.0
        onesel[64:128, h, 2 * h + 1] = 1.0
        onesel[0, h, 32 + 2 * h] = 1.0
        onesel[64, h, 32 + 2 * h + 1] = 1.0
    tri = np.triu(np.ones((128, 128), np.float32))  # keep s_loc <= t_loc
    sel8 = np.zeros((8, 4, 128), np.float32)
    for h in range(HPC):
        sel8[2 * h, h, 0:64] = -1.0       # QH rows: -gf_q
        sel8[2 * h + 1, h, 64:128] = 1.0  # KH rows: +gf_k
        sel8[2 * h + 1, h, 64] = 0.0      # zero KH row 0 (cancels r0 row)
    sel8 = sel8.reshape(8, 512)
    selb4 = np.zeros((4, 12, 128), np.float32)
    for i in range(4):
        selb4[i, i, :] = 1.0       # beta
        selb4[i, 4 + i, :] = 1.0   # gammaA
        selb4[i, 8 + i, :] = -1.0  # -a*gamma
    selb4 = selb4.reshape(4, 12 * 128)

    in_maps = []
    for c in range(NCORES):
        b, hg = c // 4, c % 4
        heads = list(range(HPC * hg, HPC * hg + HPC))
        qrows = np.concatenate([np.arange(h * D, (h + 1) * D) for h in heads])
        xTb = np.ascontiguousarray(x[b].T)  # (E, T)
        wqk_rows = np.concatenate(
            [np.concatenate([Wqkv[h * D:(h + 1) * D], Wqkv[E + h * D:E + (h + 1) * D]], 0)
             for h in heads], 0)  # (512, E)
        bqk_rows = np.stack(
            [np.concatenate([bqkv[h * D:(h + 1) * D], bqkv[E + h * D:E + (h + 1) * D]], 0)
             for h in heads], 1)  # (128, 4)
        wqkT = np.ascontiguousarray(wqk_rows.T)  # (E, 512)
        wv_rows = Wqkv[2 * E:][qrows]
        bv_rows = bqkv[2 * E:][qrows]
        wvT = np.ascontiguousarray(wv_rows.T)  # (E, 256)
        wai_rows = np.concatenate([Walpha[heads], Wimp], 0)  # (5, E)
        bai = np.concatenate([balpha[heads], np.zeros(1, np.float32)], 0)
        waiT = np.ascontiguousarray(wai_rows.T)  # (E, 5)
        woT = np.ascontiguousarray(Wout[:, qrows].T).astype(np.float16)  # (256, E)
        cstv = (0.5 * bai).reshape(5, 1).astype(np.float32)
        cstav = (A_FIT / (2.0 * kh[heads])).reshape(4, 1).astype(np.float32)
        cstlv = (A_LIN / (2.0 * kh[heads])).reshape(4, 1).astype(np.float32)
        in_maps.append({
            "xT": xTb,
            "wqk": wqkT,
            "wv": wvT,
            "wai": waiT,
            "bqk": np.ascontiguousarray(bqk_rows.astype(np.float32)),
            "bqk2": np.ascontiguousarray(
                np.concatenate([bqk_rows[0:64], bqk_rows[64:128]], 1)),
            "bvT": np.ascontiguousarray(bv_rows[None, :]),
            "wo": woT,
            "onesel": onesel,
            "cst": cstv,
            "csta": cstav,
            "cstl": cstlv,
            "tri": tri.astype(__import__("ml_dtypes").bfloat16),
            "sel8": sel8,
            "selb4": selb4,
        })

    global _last_in_maps
    _last_in_maps = in_maps
    from concourse.bass_utils import run_bass_kernel_spmd
    res = run_bass_kernel_spmd(nc, in_maps, list(range(NCORES)))

    outv = np.zeros((B, T, E), np.float32)
    for c in range(NCORES):
        outv[c // 4] += res.results[c]["out"]
    outv += bout[None, None, :]
    return outv



# revision 10
# speedup vs baseline: 1.1198x; 1.1198x over previous
"""AdaptiveGeometryAttention Trainium2 kernel (8 NeuronCores).

Sharding: core c handles batch b = c//4 and head group hg = c%4 (4 of 16 heads).
Each core computes its heads' attention and a partial out-projection (T, E);
the host sums the 4 partials per batch.

Key algebraic restructurings vs the reference:
  - The Lorentz inner product -<q_hyp, k_hyp>_L is a single K=65 matmul over
    augmented vectors [-gf*q_d | tim] x [gf*k_d | tim] with the k-side row 0
    zeroed in the broadcast selector (cancels the discarded r0 tangent row).
  - arccosh(m)^2 ~= A*tanh(sf*(m-1)+bf) + a*(m-1) + const. Constants and
    per-query additive terms cancel in softmax and are dropped; the blend is
    computed re-centered as G = psU - gamma*(F-1) so exp(G) stays in fp16
    range. The a*mdot linear term and the (1-alpha)/8 euclid scale fold into
    a second accumulated matmul (per-query column scaling of q-side operands).
  - Softmax denominators come from a ones-column appended to V; spike masking
    and 1/Z fold into one per-query scale applied to y^T before out-proj.
  - Emission is software-pipelined: psY(group g) is emitted after the score
    matmuls of group g+1 so the PE never waits on the elementwise chain.
All big matmuls run fp32r (full-rate) except attn@V / out-proj (fp16).
"""

import sys
import contextlib

sys.path.insert(0, "/opt/trn_rl_repo")

import numpy as np

B, T, E, H = 2, 1024, 1024, 16
D = 64
NCORES = 8
HPC = 4  # heads per core

# arccosh(1+x)^2 fit on x in [0, 2.2]
A_FIT = 54.32641203
S_FIT = 0.28607594936708863
B_FIT = 2.0
A_LIN = 0.8910533
# sqrt(1+w) deg-3 fit on w in [0, 0.95]
SQ3, SQ2, SQ1, SQ0 = 0.02492195, -0.10732602, 0.49672154, 1.00014421

_DEBUG = False

# host-derived scalars, set by kernel() before _build()
_S2 = _S2M2 = _SS = _THR = _BADJ = 0.0


def _build():
    from concourse import bass, mybir, tile, bacc

    F32 = mybir.dt.float32
    F32R = mybir.dt.float32r
    F16 = mybir.dt.float16
    BF16 = mybir.dt.bfloat16
    AF = mybir.ActivationFunctionType
    OP = mybir.AluOpType

    def r(ap):
        return ap.bitcast(F32R)

    nc = bacc.Bacc()

    xT = nc.declare_dram_parameter("xT", [E, T], F32R, isOutput=False)
    wqk = nc.declare_dram_parameter("wqk", [E, 512], F32R, isOutput=False)
    wv = nc.declare_dram_parameter("wv", [E, 256], F32R, isOutput=False)
    wai = nc.declare_dram_parameter("wai", [E, 5], F32R, isOutput=False)
    bqk = nc.declare_dram_parameter("bqk", [128, 4], F32, isOutput=False)
    bqk2 = nc.declare_dram_parameter("bqk2", [64, 8], F32, isOutput=False)
    bvT = nc.declare_dram_parameter("bvT", [1, 256], F32R, isOutput=False)
    wo = nc.declare_dram_parameter("wo", [256, E], F16, isOutput=False)
    onesel = nc.declare_dram_parameter("onesel", [128, 4, 40], F32R, isOutput=False)
    cst = nc.declare_dram_parameter("cst", [5, 1], F32, isOutput=False)
    csta = nc.declare_dram_parameter("csta", [4, 1], F32, isOutput=False)
    cstl = nc.declare_dram_parameter("cstl", [4, 1], F32, isOutput=False)
    tri = nc.declare_dram_parameter("tri", [128, 128], BF16, isOutput=False)
    sel8 = nc.declare_dram_parameter("sel8", [8, 512], F32R, isOutput=False)
    selb4 = nc.declare_dram_parameter("selb4", [4, 12 * 128], F32R, isOutput=False)
    out = nc.declare_dram_parameter("out", [T, E], F16, isOutput=True)
    if _DEBUG:
        d_gf = nc.declare_dram_parameter("d_gf", [8, T], F32R, isOutput=True)
        d_tim = nc.declare_dram_parameter("d_tim", [8, T], F16, isOutput=True)
        d_qhkh = nc.declare_dram_parameter("d_qhkh", [65, 8, T], F16, isOutput=True)
        d_y0 = nc.declare_dram_parameter("d_y0", [128, T], F16, isOutput=True)
        d_spk = nc.declare_dram_parameter("d_spk", [1, T], F32, isOutput=True)
        d_ba = nc.declare_dram_parameter("d_ba", [4, 3, T], F32R, isOutput=True)
        d_q0 = nc.declare_dram_parameter("d_q0", [64, T], F16, isOutput=True)
        d_k0 = nc.declare_dram_parameter("d_k0", [64, T], F16, isOutput=True)
        d_v = nc.declare_dram_parameter("d_v", [128, 8, HPC, 65], BF16, isOutput=True)

    with tile.TileContext(nc) as tc:
        ctx = contextlib.ExitStack()
        with ctx:
            main = ctx.enter_context(tc.tile_pool(name="main", bufs=1))

            # ---- persistent small inputs (vector queue: off the critical
            # path; big input loads go first on gpsimd/sync/scalar) ----
            tSEL = main.tile([128, 4, 40], F32R)
            tBQK = main.tile([128, 4], F32)
            tBQK2 = main.tile([64, 8], F32)
            tCST = main.tile([5, 1], F32)
            tCA = main.tile([4, 1], F32)
            tCL = main.tile([4, 1], F32)
            tBVT = main.tile([1, 256], F32R)
            tS8 = main.tile([8, 512], F32R)
            tS4 = main.tile([4, 12 * 128], F32R)
            tTRI = main.tile([128, 128], BF16)

            tONEf = main.tile([1, 128], F32)
            tONE = main.tile([1, 128], F32R)
            tBADJ = main.tile([128, 1], F32)
            tC625 = main.tile([4, 1], F32)

            # ---- persistent state ----
            tQ = [main.tile([64, T], F16, name=f"tQ{h}") for h in range(HPC)]
            tK = [main.tile([64, T], F16, name=f"tK{h}") for h in range(HPC)]
            tV = main.tile([128, 8, HPC, 65], BF16)
            QHKH = main.tile([65, 8, T], F16)  # slot 2h = QH_h, 2h+1 = KH_h
            Pgf = main.tile([8, T], F32R)
            Ptim = main.tile([8, T], F16)
            RW = main.tile([8, 4, T], F32)
            tA = main.tile([5, T], F32)
            beta = main.tile([4, T], F32R)
            gA = main.tile([4, T], F32R)
            ag = main.tile([4, T], F32R)
            SPK5 = main.tile([5, T], F32)
            tY0 = main.tile([128, T], F16)
            tY1 = main.tile([128, T], F16)
            tYL = [tY0, tY1]
            tWO = main.tile([128, 2, E], F16)
            # per-head score operand tiles (fp16), all 4 heads resident
            BAG = {h: (main.tile([64, T], F16, name=f"BQ{h}"),
                       main.tile([65, T], F16, name=f"AGQ{h}"),
                       main.tile([128, T], F16, name=f"GAB{h}"))
                   for h in range(HPC)}
            OO = main.tile([128, 8, E], F16)  # out-proj staging, m-major

            # ================= projection phase =================
            with tc.tile_pool(name="pin", bufs=1) as pin, \
                 tc.tile_pool(name="ppj2", bufs=1, space="PSUM") as ppj2:
                tXT = pin.tile([128, 8, T], F32R)
                tWQK = pin.tile([128, 8, 512], F32R)
                tWV = pin.tile([128, 8, 256], F32R)
                tWAI = pin.tile([128, 8, 5], F32R)
                for k in range(8):
                    nc.gpsimd.dma_start(out=tXT[:, k, :],
                                        in_=xT[k * 128:(k + 1) * 128, :])
                    nc.sync.dma_start(out=tWQK[:, k, :],
                                      in_=wqk[k * 128:(k + 1) * 128, :])
                for k in range(8):
                    nc.scalar.dma_start(out=tWAI[:, k, :],
                                        in_=wai[k * 128:(k + 1) * 128, :])
                for k in range(8):
                    nc.scalar.dma_start(out=tWV[:, k, :],
                                        in_=wv[k * 128:(k + 1) * 128, :])

                # small constants on the sync queue, after the wqk loads
                nc.sync.dma_start(out=tBQK[:], in_=bqk[:])
                nc.sync.dma_start(out=tBQK2[:], in_=bqk2[:])
                nc.sync.dma_start(out=tCST[:], in_=cst[:])
                nc.sync.dma_start(out=tCA[:], in_=csta[:])
                nc.sync.dma_start(out=tCL[:], in_=cstl[:])
                nc.sync.dma_start(out=tSEL[:], in_=onesel[:])
                nc.sync.dma_start(out=tBVT[:], in_=bvT[:])
                nc.sync.dma_start(out=tS8[:], in_=sel8[:])
                nc.sync.dma_start(out=tS4[:], in_=selb4[:])
                nc.sync.dma_start(out=tTRI[:], in_=tri[:])
                for g in range(2):
                    nc.sync.dma_start(out=tWO[:, g, :],
                                      in_=wo[g * 128:(g + 1) * 128, :])
                nc.vector.memset(tONEf[:], 1.0)
                nc.vector.tensor_copy(out=tONE[:], in_=tONEf[:])
                nc.vector.memset(tBADJ[:], _BADJ)
                nc.vector.memset(tC625[:], 0.0625)

                psNZ = ppj2.tile([40, T], F32, tag="nz")
                with tc.tile_pool(name="ppj", bufs=2, space="PSUM") as ppj:
                    for h in range(HPC):
                        ps = ppj.tile([128, T], F32, tag="psqk")
                        for k in range(8):
                            for n in range(2):
                                nc.tensor.matmul(
                                    ps[:, n * 512:(n + 1) * 512],
                                    r(tWQK[:, k, h * 128:(h + 1) * 128]),
                                    r(tXT[:, k, n * 512:(n + 1) * 512]),
                                    start=(k == 0), stop=(k == 7),
                                )
                        sq = pin.tile([128, T], F32R, tag="sq", bufs=2)
                        nc.scalar.activation(out=sq[:], in_=ps[:], func=AF.Square,
                                             bias=tBQK[:, h:h + 1])
                        nc.scalar.activation(out=tQ[h][:], in_=ps[0:64, :],
                                             func=AF.Identity,
                                             bias=tBQK2[:, h:h + 1])
                        nc.vector.tensor_scalar(tK[h][:], ps[64:128, :],
                                                tBQK2[:, 4 + h:5 + h], None,
                                                op0=OP.add)
                        for n in range(2):
                            nc.tensor.matmul(
                                psNZ[:, n * 512:(n + 1) * 512],
                                r(tSEL[:, h, 0:40]),
                                r(sq[:, n * 512:(n + 1) * 512]),
                                start=(h == 0), stop=(h == HPC - 1),
                            )

                psA = ppj2.tile([5, T], F32, tag="alpha")
                for k in range(8):
                    for n in range(2):
                        nc.tensor.matmul(
                            psA[:, n * 512:(n + 1) * 512],
                            r(tWAI[:, k, :]),
                            r(tXT[:, k, n * 512:(n + 1) * 512]),
                            start=(k == 0), stop=(k == 7),
                        )

                nc.scalar.activation(out=tA[:], in_=psA[:], func=AF.Tanh,
                                     scale=0.5, bias=tCST[0:5, :])
                nc.vector.tensor_scalar(SPK5[:], psA[0:5, :], _THR, None,
                                        op0=OP.is_gt)
                nc.scalar.activation(out=beta[:], in_=tA[0:4, :], func=AF.Identity,
                                     scale=-0.0625, bias=tC625[:])
                nc.scalar.activation(out=gA[:], in_=tA[0:4, :], func=AF.Identity,
                                     scale=tCA[:], bias=tCA[:])
                nc.scalar.activation(out=ag[:], in_=tA[0:4, :], func=AF.Identity,
                                     scale=tCL[:], bias=tCL[:])
                spk = main.tile([1, T], F32)
                nc.scalar.dma_start(out=spk[:], in_=SPK5[4:5, :])

                # ---- row quantities, chunk cc (serial chain on vector) ----
                def row_chunk(cc):
                    cl = slice(cc * 512, (cc + 1) * 512)
                    sA = RW[0:8, 0, cl]
                    sB = RW[0:8, 1, cl]
                    sC = RW[0:8, 2, cl]
                    sD = RW[0:8, 3, cl]
                    gfc = Pgf[0:8, cl]
                    timc = Ptim[0:8, cl]
                    n2 = psNZ[0:8, cl]
                    z2 = psNZ[32:40, cl]
                    nc.vector.tensor_scalar_max(sA, n2, 1e-24)
                    nc.vector.reciprocal_approx_fast(out=sB, in_=sA)      # 1/n2
                    nc.vector.tensor_mul(sC, z2, sB)                      # q2n
                    nc.vector.tensor_scalar(sA, sC, _S2M2, _S2, op0=OP.mult,
                                            op1=OP.add)
                    nc.vector.tensor_scalar_max(sA, sA, 1e-8)             # y = nu^2
                    nc.scalar.activation(out=gfc, in_=sB, func=AF.Sqrt)   # invn
                    # f = sinh(nu)/nu = 1 + y/6 + y^2/120 + y^3/5040
                    nc.vector.tensor_scalar(sD, sA, 1.0 / 5040.0, 1.0 / 120.0,
                                            op0=OP.mult, op1=OP.add)
                    nc.vector.tensor_mul(sD, sD, sA)
                    nc.vector.scalar_tensor_tensor(out=sD, in0=sD, scalar=1.0 / 6.0,
                                                   in1=sA, op0=OP.add, op1=OP.mult)
                    nc.vector.tensor_scalar_add(sA, sD, 1.0)              # f
                    nc.vector.scalar_tensor_tensor(out=gfc, in0=gfc, scalar=_SS,
                                                   in1=sA, op0=OP.mult, op1=OP.mult)
                    nc.vector.tensor_scalar(sB, sC, -_S2, _S2, op0=OP.mult,
                                            op1=OP.add)
                    nc.vector.tensor_mul(sC, sA, sA)                      # f^2
                    nc.vector.tensor_mul(sB, sC, sB)                      # w
                    nc.vector.tensor_scalar(sC, sB, SQ3, SQ2, op0=OP.mult,
                                            op1=OP.add)
                    nc.vector.tensor_mul(sC, sC, sB)
                    nc.vector.scalar_tensor_tensor(out=sC, in0=sC, scalar=SQ1,
                                                   in1=sB, op0=OP.add, op1=OP.mult)
                    nc.vector.tensor_scalar_add(timc, sC, SQ0)            # time
                    # time rows into QHKH row 64 (one DMA off the engines)
                    nc.scalar.dma_start(out=QHKH[64:65, :, cl], in_=Ptim[0:8, cl])

                row_chunk(0)

                # V projection (k-outer, PSUM-resident accumulators)
                tVonef = pin.tile([128, 32], F32)
                nc.vector.memset(tVonef[:], 1.0)
                nc.vector.tensor_copy(out=tV[:, :, :, 64:65], in_=tVonef[:])
                with tc.tile_pool(name="ppv", bufs=1, space="PSUM") as ppv:
                    psvAll = ppv.tile([128, 8, 256], F32, tag="psv")
                    for m in range(8):
                        nc.tensor.matmul(psvAll[:, m, :], r(tONE[:]), r(tBVT[:]),
                                         start=True, stop=False)
                    for k in range(8):
                        for m in range(8):
                            nc.tensor.matmul(
                                psvAll[:, m, :],
                                r(tXT[:, k, m * 128:(m + 1) * 128]),
                                r(tWV[:, k, :]),
                                start=False, stop=(k == 7),
                            )
                    for m in range(8):
                        src = psvAll[:, m, :].rearrange("p (h d) -> p h d", h=HPC)
                        nc.scalar.copy(out=tV[:, m, :, 0:64], in_=src)

                    row_chunk(1)

            # ================= attention =================
            jsl = [slice(0, 512), slice(512, 1024)]
            with tc.tile_pool(name="hp", bufs=2) as hp, \
                 tc.tile_pool(name="pps", bufs=2, space="PSUM") as pps, \
                 tc.tile_pool(name="ppy", bufs=2, space="PSUM") as ppy, \
                 tc.tile_pool(name="ppb", bufs=2, space="PSUM") as ppb:

                def prep(h, cc):
                    csl = slice(cc * 512, (cc + 1) * 512)
                    tbq, tagq, tgab = BAG[h]
                    pb = ppb.tile([128, 512], F32, tag="psb")
                    nc.tensor.matmul(pb[:], r(tS8[:, h * 128:(h + 1) * 128]),
                                     r(Pgf[:, csl]), start=True, stop=True)
                    nc.vector.tensor_mul(QHKH[0:64, 2 * h, csl], pb[0:64, :],
                                         tQ[h][:, csl])
                    nc.vector.tensor_mul(QHKH[0:64, 2 * h + 1, csl], pb[64:128, :],
                                         tK[h][:, csl])
                    pb2 = ppb.tile([128, 512], F32, tag="psb")
                    nc.tensor.matmul(pb2[0:64, :],
                                     r(tS4[:, h * 128:h * 128 + 64]),
                                     r(beta[:, csl]), start=True, stop=True)
                    nc.vector.tensor_mul(tbq[:, csl], pb2[0:64, :], tQ[h][:, csl])
                    pb3 = ppb.tile([128, 512], F32, tag="psb")
                    nc.tensor.matmul(pb3[0:65, :],
                                     r(tS4[:, (8 + h) * 128:(8 + h) * 128 + 65]),
                                     r(ag[:, csl]), start=True, stop=True)
                    nc.vector.tensor_mul(tagq[:, csl], pb3[0:65, :],
                                         QHKH[0:65, 2 * h, csl])
                    pb4 = ppb.tile([128, 512], F32, tag="psb")
                    nc.tensor.matmul(pb4[:], r(tS4[:, (4 + h) * 128:(5 + h) * 128]),
                                     r(gA[:, csl]), start=True, stop=True)
                    nc.scalar.copy(out=tgab[:, csl], in_=pb4[:])

                def scores(h, j):
                    nsb = 4 * (j + 1)
                    tbq, tagq, tgab = BAG[h]
                    PTJ = hp.tile([128, 8, 512], BF16, tag="PTJ")
                    for sb in range(nsb):
                        o = max(0, 128 * sb - 512 * j)
                        W = 512 - o
                        c0 = 512 * j + o
                        psM = pps.tile([128, 512], F32, tag="psM")
                        psU = pps.tile([128, 512], F32, tag="psU")
                        # psM and the first psU matmul share the KH stationary
                        nc.tensor.matmul(
                            psM[:, o:512],
                            QHKH[0:65, 2 * h + 1, sb * 128:(sb + 1) * 128],
                            QHKH[0:65, 2 * h, c0:c0 + W],
                            start=True, stop=True)
                        nc.tensor.matmul(
                            psU[:, o:512],
                            QHKH[0:65, 2 * h + 1, sb * 128:(sb + 1) * 128],
                            tagq[:, c0:c0 + W], start=True, stop=False)
                        nc.tensor.matmul(
                            psU[:, o:512],
                            tK[h][:, sb * 128:(sb + 1) * 128],
                            tbq[:, c0:c0 + W], start=False, stop=True)
                        F = hp.tile([128, 512], F16, tag="F", bufs=3)
                        nc.scalar.activation(out=F[:, o:512], in_=psM[:, o:512],
                                             func=AF.Tanh, scale=S_FIT,
                                             bias=tBADJ[:])
                        G = hp.tile([128, 512], F16, tag="G", bufs=3)
                        # G = (F - 1) * gab  (re-centered: shifts scores by
                        # +gamma(q), a per-query constant that softmax cancels)
                        nc.gpsimd.tensor_mul(G[:, o:512], F[:, o:512],
                                             tgab[:, c0:c0 + W])
                        nc.vector.scalar_tensor_tensor(
                            out=G[:, o:512], in0=G[:, o:512], scalar=-1.0,
                            in1=psU[:, o:512], op0=OP.mult, op1=OP.add)
                        nc.scalar.activation(out=PTJ[:, sb, o:512], in_=G[:, o:512],
                                             func=AF.Exp)
                        if sb >= 4 * j:
                            nc.gpsimd.tensor_mul(PTJ[:, sb, o:o + 128],
                                                 PTJ[:, sb, o:o + 128], tTRI[:, :])
                    return PTJ

                def finishA(h, j, PTJ):
                    nsb = 4 * (j + 1)
                    psY = ppy.tile([65, 512], F32, tag="psY")
                    for sb in range(nsb):
                        o = max(0, 128 * sb - 512 * j)
                        nc.tensor.matmul(
                            psY[:, o:512],
                            tV[:, sb, h, :],
                            PTJ[:, sb, o:512],
                            start=(sb == 0), stop=(sb == nsb - 1))
                    zrow = hp.tile([1, 512], F32, tag="zrow", bufs=1)
                    nc.vector.tensor_copy(out=zrow[:], in_=psY[64:65, :])
                    rz = hp.tile([1, 512], F32, tag="rz")
                    nc.vector.reciprocal_approx_fast(out=rz[:], in_=zrow[:])
                    cs = hp.tile([1, 512], F32R, tag="cs")
                    nc.vector.tensor_mul(cs[:], rz[:], spk[0:1, jsl[j]])
                    return psY, cs

                def finishB(h, j, psY, cs):
                    psc = pps.tile([128, 512], F32, tag="psM")
                    nc.tensor.matmul(psc[0:64, :], r(tONE[:, 0:64]), r(cs[:]),
                                     start=True, stop=True)
                    cbs = hp.tile([64, 512], F32, tag="cbs", bufs=1)
                    nc.scalar.copy(out=cbs[:], in_=psc[0:64, :])
                    g = h // 2
                    rows = slice((h % 2) * 64, (h % 2) * 64 + 64)
                    nc.vector.tensor_mul(tYL[g][rows, jsl[j]], psY[0:64, :],
                                         cbs[:])

                def outproj(ms, eng):
                    for m in ms:
                        po = pps.tile([128, 512], F32, tag="psM")
                        po2 = pps.tile([128, 512], F32, tag="psU")
                        for ne, pot in ((0, po), (1, po2)):
                            for g in range(2):
                                nc.tensor.matmul(
                                    pot[:],
                                    tYL[g][:, m * 128:(m + 1) * 128],
                                    tWO[:, g, ne * 512:(ne + 1) * 512],
                                    start=(g == 0), stop=(g == 1))
                        nc.scalar.copy(out=OO[:, m, 0:512], in_=po[:])
                        nc.vector.tensor_copy(out=OO[:, m, 512:1024], in_=po2[:])
                    m0, m1 = ms[0], ms[-1] + 1
                    eng.dma_start(
                        out=out[m0 * 128:m1 * 128, :].rearrange(
                            "(m p) e -> p m e", p=128),
                        in_=OO[:, m0:m1, :])

                order = [(0, 0), (1, 0), (2, 0), (3, 0),
                         (0, 1), (1, 1), (2, 1), (3, 1)]
                preps = {0: [(2, 0)], 1: [(3, 0)], 2: [(0, 1)], 3: [(1, 1)],
                         4: [(2, 1)], 5: [(3, 1)]}
                prep(0, 0)
                prep(1, 0)
                PTs = {}
                Ys = {}
                for gi, (h, j) in enumerate(order):
                    PTs[gi] = scores(h, j)
                    for (ph, pc) in preps.get(gi, []):
                        prep(ph, pc)
                    if gi >= 1:
                        h1, j1 = order[gi - 1]
                        Ys[gi - 1] = finishA(h1, j1, PTs.pop(gi - 1))
                    if gi >= 2:
                        h2, j2 = order[gi - 2]
                        finishB(h2, j2, *Ys.pop(gi - 2))
                        if gi == 5:
                            # query block 0 fully in tY -> stream first half
                            # of the out-projection under the j=1 scores
                            outproj(range(0, 4), nc.sync)
                h1, j1 = order[7]
                Ys[7] = finishA(h1, j1, PTs.pop(7))
                h2, j2 = order[6]
                finishB(h2, j2, *Ys.pop(6))
                finishB(h1, j1, *Ys.pop(7))

                if _DEBUG:
                    nc.sync.dma_start(out=d_y0[:], in_=tY0[:])
                    nc.sync.dma_start(out=d_gf[:], in_=Pgf[:])
                    nc.sync.dma_start(out=d_tim[:], in_=Ptim[:])
                    nc.sync.dma_start(out=d_qhkh[:], in_=QHKH[:])
                    nc.sync.dma_start(out=d_spk[:], in_=spk[:])
                    nc.sync.dma_start(out=d_ba[:, 0, :], in_=beta[:])
                    nc.sync.dma_start(out=d_ba[:, 1, :], in_=gA[:])
                    nc.sync.dma_start(out=d_ba[:, 2, :], in_=ag[:])
                    nc.sync.dma_start(out=d_q0[:], in_=tQ[0][:])
                    nc.sync.dma_start(out=d_k0[:], in_=tK[0][:])
                    nc.sync.dma_start(out=d_v[:], in_=tV[:])

                # ---- out projection: second half (query block 1) ----
                outproj(range(4, 8), nc.gpsimd)

    nc.finalize()
    return nc


_NC_CACHE = None


def _np_sigmoid(x):
    return 1.0 / (1.0 + np.exp(-x))


def kernel(**inputs):
    global _NC_CACHE, _S2, _S2M2, _SS, _THR, _BADJ
    x = np.asarray(inputs["x"], np.float32)
    Wqkv = np.asarray(inputs["Wqkv"], np.float32)
    bqkv = np.asarray(inputs["bqkv"], np.float32)
    Wout = np.asarray(inputs["Wout"], np.float32)
    bout = np.asarray(inputs["bout"], np.float32)
    Wimp = np.asarray(inputs["Wimp"], np.float32)
    bimp = np.asarray(inputs["bimp"], np.float32)
    Walpha = np.asarray(inputs["Walpha"], np.float32)
    balpha = np.asarray(inputs["balpha"], np.float32)
    spike_threshold = float(np.asarray(inputs["spike_threshold"]))
    log_k = np.asarray(inputs["log_k"], np.float32)
    qk_scale = float(np.asarray(inputs["qk_scale"]))

    s = _np_sigmoid(qk_scale) * 1.5
    kh = np.log1p(np.exp(log_k.astype(np.float64))) + 1e-6
    _S2 = float(s * s)
    _S2M2 = float(-2.0 * s * s)
    _SS = float(s)
    _THR = float(np.log(spike_threshold / (1.0 - spike_threshold)) - bimp[0])
    _BADJ = float(B_FIT - S_FIT)  # tanh(sf*M + (bf - sf)) = tanh(sf*(M-1)+bf)

    if _NC_CACHE is None:
        _NC_CACHE = _build()
    nc = _NC_CACHE

    onesel = np.zeros((128, 4, 40), np.float32)
    for h in range(HPC):
        onesel[0:64, h, 2 * h] = 1.0
        onesel[64:128, h, 2 * h + 1] = 1.0
        onesel[0, h, 32 + 2 * h] = 1.0
        onesel[64, h, 32 + 2 * h + 1] = 1.0
    tri = np.triu(np.ones((128, 128), np.float32))  # keep s_loc <= t_loc
    sel8 = np.zeros((8, 4, 128), np.float32)
    for h in range(HPC):
        sel8[2 * h, h, 0:64] = -1.0       # QH rows: -gf_q
        sel8[2 * h + 1, h, 64:128] = 1.0  # KH rows: +gf_k
        sel8[2 * h + 1, h, 64] = 0.0      # zero KH row 0 (cancels r0 row)
    sel8 = sel8.reshape(8, 512)
    selb4 = np.zeros((4, 12, 128), np.float32)
    for i in range(4):
        selb4[i, i, :] = 1.0       # beta
        selb4[i, 4 + i, :] = 1.0   # gammaA
        selb4[i, 8 + i, :] = -1.0  # -a*gamma
    selb4 = selb4.reshape(4, 12 * 128)

    in_maps = []
    for c in range(NCORES):
        b, hg = c // 4, c % 4
        heads = list(range(HPC * hg, HPC * hg + HPC))
        qrows = np.concatenate([np.arange(h * D, (h + 1) * D) for h in heads])
        xTb = np.ascontiguousarray(x[b].T)  # (E, T)
        wqk_rows = np.concatenate(
            [np.concatenate([Wqkv[h * D:(h + 1) * D], Wqkv[E + h * D:E + (h + 1) * D]], 0)
             for h in heads], 0)  # (512, E)
        bqk_rows = np.stack(
            [np.concatenate([bqkv[h * D:(h + 1) * D], bqkv[E + h * D:E + (h + 1) * D]], 0)
             for h in heads], 1)  # (128, 4)
        wqkT = np.ascontiguousarray(wqk_rows.T)  # (E, 512)
        wv_rows = Wqkv[2 * E:][qrows]
        bv_rows = bqkv[2 * E:][qrows]
        wvT = np.ascontiguousarray(wv_rows.T)  # (E, 256)
        wai_rows = np.concatenate([Walpha[heads], Wimp], 0)  # (5, E)
        bai = np.concatenate([balpha[heads], np.zeros(1, np.float32)], 0)
        waiT = np.ascontiguousarray(wai_rows.T)  # (E, 5)
        woT = np.ascontiguousarray(Wout[:, qrows].T).astype(np.float16)  # (256, E)
        cstv = (0.5 * bai).reshape(5, 1).astype(np.float32)
        cstav = (A_FIT / (2.0 * kh[heads])).reshape(4, 1).astype(np.float32)
        cstlv = (A_LIN / (2.0 * kh[heads])).reshape(4, 1).astype(np.float32)
        in_maps.append({
            "xT": xTb,
            "wqk": wqkT,
            "wv": wvT,
            "wai": waiT,
            "bqk": np.ascontiguousarray(bqk_rows.astype(np.float32)),
            "bqk2": np.ascontiguousarray(
                np.concatenate([bqk_rows[0:64], bqk_rows[64:128]], 1)),
            "bvT": np.ascontiguousarray(bv_rows[None, :]),
            "wo": woT,
            "onesel": onesel,
            "cst": cstv,
            "csta": cstav,
            "cstl": cstlv,
            "tri": tri.astype(__import__("ml_dtypes").bfloat16),
            "sel8": sel8,
            "selb4": selb4,
        })

    global _last_in_maps
    _last_in_maps = in_maps
    from concourse.bass_utils import run_bass_kernel_spmd
    res = run_bass_kernel_spmd(nc, in_maps, list(range(NCORES)))

    outv = np.zeros((B, T, E), np.float32)
    for c in range(NCORES):
        outv[c // 4] += res.results[c]["out"].astype(np.float32)
    outv += bout[None, None, :]
    return outv

